# revision 22
# baseline (speedup 1.0000x reference)
"""Trainium2 Bass kernel for the gnn_message_passing problem.

Math refactor: the reference computes
    kernel[z,i,j] = einsum('zk,kij->zij', Rk*Yk, cg) * nc0[i,j]
with Rk = R @ rf_mix.T (rank 6 over paths) and Yk = Y.T @ ylm_mix.T
(rank 9 over l,m).  Rk*Yk has rank <= 54 over k, so the K=1024
contraction folds into one constant matrix
    M[p*9+l, ij] = sum_k rf[k,p] * ylm_s[k,l] * cg[k,ij] * nc0[ij]
and the per-point factor is B[p*9+l, z] = (R+b2)[z,p] * Y'[z,l] -- a
rank-54 stack built from 15 cheap per-point values (6 radial-MLP paths,
9 scaled sh monomials).  B is a pure per-point prefactor (~2% of the
problem's FLOPs) and is prepared host-side in float64 alongside the
constant folds; the device runs the Clebsch-Gordan contraction itself
(98% of FLOPs): per 128-point tile, out = B_tile^T @ M as a k=54 bf16
matmul into f32 PSUM.

The kernel is memory-bound on the output store.  The DMA fabric is a
single ~360 GB/s resource, so bytes are the floor: storing f32 costs
142 us/core.  The output therefore ships as bf16 (25.6 MB/core,
~71 us) and the host widens bf16->f32 during unshard; bf16 rounding
adds <0.5% relative error against the 2e-2 gate.

Per-core pipeline (12500 points = 98 tiles of 128):
  - two merged head loads ([M_lo | B tile 0] and [M_hi | B tiles 1-5])
    on the two lowest-latency trigger paths gate the pipeline ~3.1 us
    in; the remaining B arrives in 4 bulk chunks that all fit inside
    the fill window, so loads cost no store-stream time.
  - per tile: 2 matmuls (B_tile stationary, M halves moving) into two
    INDEPENDENT one-bank PSUM rotations (4 buffers each), so a matmul
    waits only on its own half's drain; the PSUM->SBUF bf16 convert
    splits ACT (first half) + DVE (second half); one 256 KB store,
    alternating SP-HWDGE / Pool-SWDGE triggers so no sequencer or DGE
    saturates.  The first two tiles use quarter-width matmuls and the
    first three use half-column stores (both halves on one trigger,
    alternating sync/gpsimd) to shorten the fill-phase critical path.

Distribution: data-parallel over z across 8 NeuronCores; constants
replicated.  Full inputs in, full output out.
"""

import numpy as np
import ml_dtypes

import concourse.bass as bass
import concourse.tile as tile
from concourse import bacc, mybir
from concourse.bass_utils import run_bass_kernel_spmd

F32 = mybir.dt.float32
BF16 = mybir.dt.bfloat16

# Problem shape (hardcoded per contract)
Z, KDIM, DO, DI, NPATH, H = 100000, 1024, 32, 32, 6, 128
IJ = DO * DI                      # 1024
PL = NPATH * 9                    # 54 (path x lm)
NCORES = 8
ZC = Z // NCORES                  # 12500 points per core
NT = 98                           # tiles of 128 (12544 >= 12500)
ZPAD = NT * 128

HEAD = 6                          # B tiles packed into the two head loads
CHUNKS = ((6, 24), (30, 24), (54, 22), (76, 22))
SPLIT_MM = 2                      # leading tiles with quarter-width matmuls
HALFCOL = 3                       # leading tiles with half-column stores

# Real spherical harmonic constants (l=0,1,2), folded into M host-side
C0 = 0.28209479177387814
C1 = 0.4886025119029199
C2A = 1.0925484305920792
C2B = 0.31539156525252005
C2C = 0.5462742152960396
YLM_SCALE = np.array([C0, C1, C1, C1, C2A, C2A, C2B, C2A, C2C], dtype=np.float64)

_CACHE = {}


def _build_program():
    nc = bacc.Bacc("TRN2", target_bir_lowering=False, debug=False,
                   num_devices=NCORES)

    hd0 = nc.dram_tensor("hd0", [PL, 512 + 128], BF16,
                         kind="ExternalInput").ap()
    hd1 = nc.dram_tensor("hd1", [PL, 512 + (HEAD - 1) * 128], BF16,
                         kind="ExternalInput").ap()
    bds = [nc.dram_tensor(f"b{i}d", [PL, n * 128], BF16,
                          kind="ExternalInput").ap()
           for i, (_, n) in enumerate(CHUNKS)]
    out = nc.dram_tensor("out", [ZC, IJ], BF16, kind="ExternalOutput").ap()

    with tile.TileContext(nc) as tc:
        with tc.tile_pool(name="const", bufs=1) as cpool, \
             tc.tile_pool(name="kpsA", bufs=4, space="PSUM") as kpoolA, \
             tc.tile_pool(name="kpsB", bufs=4, space="PSUM") as kpoolB, \
             tc.tile_pool(name="kout", bufs=12) as spool:
            h0_sb = cpool.tile([PL, 512 + 128], BF16)
            h1_sb = cpool.tile([PL, 512 + (HEAD - 1) * 128], BF16)
            b_sb = cpool.tile([PL, (NT - HEAD) * 128], BF16)
            mhalves = (h0_sb[:, 0:512], h1_sb[:, 0:512])

            def bt(t):
                if t == 0:
                    return h0_sb[:, 512:640]
                if t < HEAD:
                    return h1_sb[:, 512 + (t - 1) * 128:512 + t * 128]
                return b_sb[:, (t - HEAD) * 128:(t - HEAD + 1) * 128]

            def chunk_load(eng, ci):
                t0, n = CHUNKS[ci]
                c0 = (t0 - HEAD) * 128
                eng.dma_start(b_sb[:, c0:c0 + n * 128], bds[ci][:])

            nc.sync.dma_start(h0_sb[:], hd0[:])
            nc.gpsimd.dma_start(h1_sb[:], hd1[:])
            chunk_load(nc.scalar, 0)
            chunk_load(nc.gpsimd, 1)
            chunk_load(nc.sync, 2)
            chunk_load(nc.scalar, 3)

            for t in range(NT):
                bT = bt(t)
                # independent one-bank PSUM rotations per output half: tile
                # t+4's first matmul can start as soon as tile t's first
                # half-copy (not both) has drained its bank
                kpsA = kpoolA.tile([128, 512], F32, tag="kA")
                kpsB = kpoolB.tile([128, 512], F32, tag="kB")
                halves_ps = (kpsA, kpsB)
                if t < SPLIT_MM:
                    for q in range(4):
                        mq = mhalves[q // 2][:, (q % 2) * 256:
                                             (q % 2 + 1) * 256]
                        nc.tensor.matmul(
                            halves_ps[q // 2][:, (q % 2) * 256:
                                              (q % 2 + 1) * 256],
                            bT, mq, start=True, stop=True)
                else:
                    nc.tensor.matmul(kpsA[:], bT, mhalves[0],
                                     start=True, stop=True)
                    nc.tensor.matmul(kpsB[:], bT, mhalves[1],
                                     start=True, stop=True)
                k_sb = spool.tile([128, IJ], BF16, tag="k_sb")
                nc.scalar.copy(k_sb[:, 0:512], kpsA[:])
                nc.vector.tensor_copy(k_sb[:, 512:1024], kpsB[:])
                zt = t * 128
                rows = min(128, ZC - zt)
                eng = nc.sync if t % 2 == 1 else nc.gpsimd
                if t < HALFCOL:
                    # both halves of a leading tile ride the same trigger,
                    # alternating sync/gpsimd per tile (best measured fill)
                    he = nc.sync if t % 2 == 0 else nc.gpsimd
                    he.dma_start(out[zt:zt + rows, 0:512],
                                 k_sb[0:rows, 0:512])
                    he.dma_start(out[zt:zt + rows, 512:1024],
                                 k_sb[0:rows, 512:1024])
                else:
                    eng.dma_start(out[zt:zt + rows, :], k_sb[0:rows, :])
    nc.compile()
    return nc


def _get_program():
    if "nc" not in _CACHE:
        _CACHE["nc"] = _build_program()
    return _CACHE["nc"]


def _host_b(rp):
    """B stack for one core's padded points: B[p*9+l, z] =
    (R[z] + b2)[p] * Y'[z, l], computed in float64, cast to bf16.
    Y' carries the raw monomials; the C-coefficients are folded into M."""
    pts = rp.astype(np.float64)
    x, y, z = pts[:, 0], pts[:, 1], pts[:, 2]
    r2 = x * x + y * y + z * z
    saf = np.where(r2 > 0, r2, 1.0)
    inv_r = 1.0 / np.sqrt(saf)
    inv2 = 1.0 / saf
    radii = r2 * inv_r
    h = np.maximum(radii[:, None] * _CACHE["W1"][0][None, :]
                   + _CACHE["b1"][None, :], 0.0)
    R = h @ _CACHE["W2"] + _CACHE["b2"][None, :]
    yp = np.stack([
        np.ones_like(x), y * inv_r, z * inv_r, x * inv_r,
        x * y * inv2, y * z * inv2, (3.0 * z * z - r2) * inv2,
        x * z * inv2, (x * x - y * y) * inv2,
    ], axis=1)                                            # [z, 9]
    b = (R[:, :, None] * yp[:, None, :]).reshape(-1, PL)  # [z, 54]
    return np.ascontiguousarray(b.T).astype(ml_dtypes.bfloat16)


def _host_prep(r, W1, b1, W2, b2, cg, ylm_mix, rf_mix, norm_coef):
    r = np.asarray(r, dtype=np.float32)
    W1 = np.asarray(W1, dtype=np.float32)
    b1 = np.asarray(b1, dtype=np.float32)
    W2 = np.asarray(W2, dtype=np.float32)
    b2 = np.asarray(b2, dtype=np.float32)
    cg = np.asarray(cg, dtype=np.float32)
    ylm_mix = np.asarray(ylm_mix, dtype=np.float32)
    rf_mix = np.asarray(rf_mix, dtype=np.float32)
    norm_coef = np.asarray(norm_coef, dtype=np.float32)
    _CACHE["W1"] = W1.astype(np.float64)
    _CACHE["b1"] = b1.astype(np.float64)
    _CACHE["W2"] = W2.astype(np.float64)
    _CACHE["b2"] = b2.astype(np.float64)

    # Fold the constant k-contraction: M[p*9+l, ij] =
    #   sum_k rf[k,p] * (ylm[k,l]*scale_l) * cg[k,ij], times nc0[ij]
    ylm_s = ylm_mix.astype(np.float64) * YLM_SCALE[None, :]
    w54 = (rf_mix.astype(np.float64)[:, :, None]
           * ylm_s[:, None, :]).reshape(KDIM, PL)
    mfold = w54.T @ cg.astype(np.float64).reshape(KDIM, IJ)
    mfold *= norm_coef[:, :, 0].astype(np.float64).reshape(1, IJ)
    mn = mfold.astype(ml_dtypes.bfloat16)

    in_maps = []
    for c in range(NCORES):
        rs = r[c * ZC:(c + 1) * ZC]
        rp = np.empty((ZPAD, 3), dtype=np.float32)
        rp[:ZC] = rs
        rp[ZC:] = np.array([1.0, 0.0, 0.0], dtype=np.float32)
        bfull = _host_b(rp)                               # [54, ZPAD] bf16
        h0 = np.concatenate([mn[:, 0:512], bfull[:, 0:128]], axis=1)
        h1 = np.concatenate([mn[:, 512:1024],
                             bfull[:, 128:HEAD * 128]], axis=1)
        m = {"hd0": np.ascontiguousarray(h0), "hd1": np.ascontiguousarray(h1)}
        for i, (t0, n) in enumerate(CHUNKS):
            m[f"b{i}d"] = np.ascontiguousarray(
                bfull[:, t0 * 128:(t0 + n) * 128])
        in_maps.append(m)
    return in_maps


def _run_device(in_maps, trace=False, **kw):
    nc = _get_program()
    return run_bass_kernel_spmd(nc, in_maps, core_ids=list(range(NCORES)),
                                trace=trace, **kw)


def kernel(r, W1, b1, W2, b2, cg, ylm_mix, rf_mix, norm_coef):
    r = np.asarray(r, dtype=np.float32)
    norm_coef_f = np.asarray(norm_coef, dtype=np.float32)
    in_maps = _host_prep(r, W1, b1, W2, b2, cg, ylm_mix, rf_mix, norm_coef_f)
    res = _run_device(in_maps)
    out = np.concatenate(
        [np.asarray(res.results[c]["out"]).astype(np.float32)
         for c in range(NCORES)], axis=0)

    # points with exactly zero radius use norm_coef[..., 1] instead of [..., 0]
    x, y, z = r[:, 0], r[:, 1], r[:, 2]
    r2 = (x * x + y * y) + z * z
    zero = r2 == np.float32(0.0)
    if np.any(zero):
        scale = (norm_coef_f[:, :, 1].astype(np.float64)
                 / norm_coef_f[:, :, 0].astype(np.float64)).reshape(1, IJ)
        out[zero] = (out[zero].astype(np.float64) * scale).astype(np.float32)

    return out.reshape(Z, DO, DI)


# revision 24
# speedup vs baseline: 1.0028x; 1.0028x over previous
"""Trainium2 Bass kernel for the gnn_message_passing problem.

Math refactor: the reference computes
    kernel[z,i,j] = einsum('zk,kij->zij', Rk*Yk, cg) * nc0[i,j]
with Rk = R @ rf_mix.T (rank 6 over paths) and Yk = Y.T @ ylm_mix.T
(rank 9 over l,m).  Rk*Yk has rank <= 54 over k, so the K=1024
contraction folds into one constant matrix
    M[p*9+l, ij] = sum_k rf[k,p] * ylm_s[k,l] * cg[k,ij] * nc0[ij]
and the per-point factor is B[p*9+l, z] = (R+b2)[z,p] * Y'[z,l] -- a
rank-54 stack built from 15 cheap per-point values (6 radial-MLP paths,
9 scaled sh monomials).  B is a pure per-point prefactor (~2% of the
problem's FLOPs) and is prepared host-side in float64 alongside the
constant folds; the device runs the Clebsch-Gordan contraction itself
(98% of FLOPs): per 128-point tile, out = B_tile^T @ M as a k=54 bf16
matmul into f32 PSUM.

The kernel is memory-bound on the output store.  The DMA fabric is a
single ~360 GB/s resource, so bytes are the floor: storing f32 costs
142 us/core.  The output therefore ships as bf16 (25.6 MB/core,
~71 us) and the host widens bf16->f32 during unshard; bf16 rounding
adds <0.5% relative error against the 2e-2 gate.

Per-core pipeline (12500 points = 98 tiles of 128):
  - two merged head loads ([M_lo | B tile 0] and [M_hi | B tiles 1-5])
    on the two lowest-latency trigger paths gate the pipeline ~3.1 us
    in; the remaining B arrives in 4 bulk chunks that all fit inside
    the fill window, so loads cost no store-stream time.
  - per tile: 2 matmuls (B_tile stationary, M halves moving) into two
    INDEPENDENT one-bank PSUM rotations (4 buffers each), so a matmul
    waits only on its own half's drain; the PSUM->SBUF bf16 convert
    splits ACT (first half) + DVE (second half); one 256 KB store,
    alternating SP-HWDGE / Pool-SWDGE triggers so no sequencer or DGE
    saturates.  The first two tiles use quarter-width matmuls and the
    first three use half-column stores (both halves on one trigger,
    alternating sync/gpsimd) to shorten the fill-phase critical path.

Distribution: data-parallel over z across 8 NeuronCores; constants
replicated.  Full inputs in, full output out.
"""

import numpy as np
import ml_dtypes

import concourse.bass as bass
import concourse.tile as tile
from concourse import bacc, mybir
from concourse.bass_utils import run_bass_kernel_spmd

F32 = mybir.dt.float32
BF16 = mybir.dt.bfloat16

# Problem shape (hardcoded per contract)
Z, KDIM, DO, DI, NPATH, H = 100000, 1024, 32, 32, 6, 128
IJ = DO * DI                      # 1024
PL = NPATH * 9                    # 54 (path x lm)
NCORES = 8
ZC = Z // NCORES                  # 12500 points per core
NT = 98                           # tiles of 128 (12544 >= 12500)
ZPAD = NT * 128

HEAD = 6                          # B tiles packed into the two head loads
CHUNKS = ((6, 24), (30, 24), (54, 22), (76, 22))
SPLIT_MM = 2                      # leading tiles with quarter-width matmuls
HALFCOL = 5                       # leading tiles with half-column stores

# Real spherical harmonic constants (l=0,1,2), folded into M host-side
C0 = 0.28209479177387814
C1 = 0.4886025119029199
C2A = 1.0925484305920792
C2B = 0.31539156525252005
C2C = 0.5462742152960396
YLM_SCALE = np.array([C0, C1, C1, C1, C2A, C2A, C2B, C2A, C2C], dtype=np.float64)

_CACHE = {}


def _build_program():
    nc = bacc.Bacc("TRN2", target_bir_lowering=False, debug=False,
                   num_devices=NCORES)

    hd0 = nc.dram_tensor("hd0", [PL, 512 + 128], BF16,
                         kind="ExternalInput").ap()
    hd1 = nc.dram_tensor("hd1", [PL, 512 + (HEAD - 1) * 128], BF16,
                         kind="ExternalInput").ap()
    bds = [nc.dram_tensor(f"b{i}d", [PL, n * 128], BF16,
                          kind="ExternalInput").ap()
           for i, (_, n) in enumerate(CHUNKS)]
    out = nc.dram_tensor("out", [ZC, IJ], BF16, kind="ExternalOutput").ap()

    with tile.TileContext(nc) as tc:
        with tc.tile_pool(name="const", bufs=1) as cpool, \
             tc.tile_pool(name="kpsA", bufs=4, space="PSUM") as kpoolA, \
             tc.tile_pool(name="kpsB", bufs=4, space="PSUM") as kpoolB, \
             tc.tile_pool(name="kout", bufs=12) as spool:
            h0_sb = cpool.tile([PL, 512 + 128], BF16)
            h1_sb = cpool.tile([PL, 512 + (HEAD - 1) * 128], BF16)
            b_sb = cpool.tile([PL, (NT - HEAD) * 128], BF16)
            mhalves = (h0_sb[:, 0:512], h1_sb[:, 0:512])

            def bt(t):
                if t == 0:
                    return h0_sb[:, 512:640]
                if t < HEAD:
                    return h1_sb[:, 512 + (t - 1) * 128:512 + t * 128]
                return b_sb[:, (t - HEAD) * 128:(t - HEAD + 1) * 128]

            def chunk_load(eng, ci):
                t0, n = CHUNKS[ci]
                c0 = (t0 - HEAD) * 128
                eng.dma_start(b_sb[:, c0:c0 + n * 128], bds[ci][:])

            nc.sync.dma_start(h0_sb[:], hd0[:])
            nc.gpsimd.dma_start(h1_sb[:], hd1[:])
            chunk_load(nc.scalar, 0)
            chunk_load(nc.gpsimd, 1)
            chunk_load(nc.sync, 2)
            chunk_load(nc.scalar, 3)

            for t in range(NT):
                bT = bt(t)
                # independent one-bank PSUM rotations per output half: tile
                # t+4's first matmul can start as soon as tile t's first
                # half-copy (not both) has drained its bank
                kpsA = kpoolA.tile([128, 512], F32, tag="kA")
                kpsB = kpoolB.tile([128, 512], F32, tag="kB")
                halves_ps = (kpsA, kpsB)
                if t < SPLIT_MM:
                    for q in range(4):
                        mq = mhalves[q // 2][:, (q % 2) * 256:
                                             (q % 2 + 1) * 256]
                        nc.tensor.matmul(
                            halves_ps[q // 2][:, (q % 2) * 256:
                                              (q % 2 + 1) * 256],
                            bT, mq, start=True, stop=True)
                else:
                    nc.tensor.matmul(kpsA[:], bT, mhalves[0],
                                     start=True, stop=True)
                    nc.tensor.matmul(kpsB[:], bT, mhalves[1],
                                     start=True, stop=True)
                k_sb = spool.tile([128, IJ], BF16, tag="k_sb")
                nc.scalar.copy(k_sb[:, 0:512], kpsA[:])
                nc.vector.tensor_copy(k_sb[:, 512:1024], kpsB[:])
                zt = t * 128
                rows = min(128, ZC - zt)
                eng = nc.sync if t % 2 == 1 else nc.gpsimd
                if t < HALFCOL:
                    # leading tiles store as halves on split triggers:
                    # first half Pool-SWDGE, second half SP-HWDGE (best
                    # measured fill - the two dispatch paths pipeline)
                    nc.gpsimd.dma_start(out[zt:zt + rows, 0:512],
                                        k_sb[0:rows, 0:512])
                    nc.sync.dma_start(out[zt:zt + rows, 512:1024],
                                      k_sb[0:rows, 512:1024])
                else:
                    eng.dma_start(out[zt:zt + rows, :], k_sb[0:rows, :])
    nc.compile()
    return nc


def _get_program():
    if "nc" not in _CACHE:
        _CACHE["nc"] = _build_program()
    return _CACHE["nc"]


def _host_b(rp):
    """B stack for one core's padded points: B[p*9+l, z] =
    (R[z] + b2)[p] * Y'[z, l], computed in float64, cast to bf16.
    Y' carries the raw monomials; the C-coefficients are folded into M."""
    pts = rp.astype(np.float64)
    x, y, z = pts[:, 0], pts[:, 1], pts[:, 2]
    r2 = x * x + y * y + z * z
    saf = np.where(r2 > 0, r2, 1.0)
    inv_r = 1.0 / np.sqrt(saf)
    inv2 = 1.0 / saf
    radii = r2 * inv_r
    h = np.maximum(radii[:, None] * _CACHE["W1"][0][None, :]
                   + _CACHE["b1"][None, :], 0.0)
    R = h @ _CACHE["W2"] + _CACHE["b2"][None, :]
    yp = np.stack([
        np.ones_like(x), y * inv_r, z * inv_r, x * inv_r,
        x * y * inv2, y * z * inv2, (3.0 * z * z - r2) * inv2,
        x * z * inv2, (x * x - y * y) * inv2,
    ], axis=1)                                            # [z, 9]
    b = (R[:, :, None] * yp[:, None, :]).reshape(-1, PL)  # [z, 54]
    return np.ascontiguousarray(b.T).astype(ml_dtypes.bfloat16)


def _host_prep(r, W1, b1, W2, b2, cg, ylm_mix, rf_mix, norm_coef):
    r = np.asarray(r, dtype=np.float32)
    W1 = np.asarray(W1, dtype=np.float32)
    b1 = np.asarray(b1, dtype=np.float32)
    W2 = np.asarray(W2, dtype=np.float32)
    b2 = np.asarray(b2, dtype=np.float32)
    cg = np.asarray(cg, dtype=np.float32)
    ylm_mix = np.asarray(ylm_mix, dtype=np.float32)
    rf_mix = np.asarray(rf_mix, dtype=np.float32)
    norm_coef = np.asarray(norm_coef, dtype=np.float32)
    _CACHE["W1"] = W1.astype(np.float64)
    _CACHE["b1"] = b1.astype(np.float64)
    _CACHE["W2"] = W2.astype(np.float64)
    _CACHE["b2"] = b2.astype(np.float64)

    # Fold the constant k-contraction: M[p*9+l, ij] =
    #   sum_k rf[k,p] * (ylm[k,l]*scale_l) * cg[k,ij], times nc0[ij]
    ylm_s = ylm_mix.astype(np.float64) * YLM_SCALE[None, :]
    w54 = (rf_mix.astype(np.float64)[:, :, None]
           * ylm_s[:, None, :]).reshape(KDIM, PL)
    mfold = w54.T @ cg.astype(np.float64).reshape(KDIM, IJ)
    mfold *= norm_coef[:, :, 0].astype(np.float64).reshape(1, IJ)
    mn = mfold.astype(ml_dtypes.bfloat16)

    in_maps = []
    for c in range(NCORES):
        rs = r[c * ZC:(c + 1) * ZC]
        rp = np.empty((ZPAD, 3), dtype=np.float32)
        rp[:ZC] = rs
        rp[ZC:] = np.array([1.0, 0.0, 0.0], dtype=np.float32)
        bfull = _host_b(rp)                               # [54, ZPAD] bf16
        h0 = np.concatenate([mn[:, 0:512], bfull[:, 0:128]], axis=1)
        h1 = np.concatenate([mn[:, 512:1024],
                             bfull[:, 128:HEAD * 128]], axis=1)
        m = {"hd0": np.ascontiguousarray(h0), "hd1": np.ascontiguousarray(h1)}
        for i, (t0, n) in enumerate(CHUNKS):
            m[f"b{i}d"] = np.ascontiguousarray(
                bfull[:, t0 * 128:(t0 + n) * 128])
        in_maps.append(m)
    return in_maps


def _run_device(in_maps, trace=False, **kw):
    nc = _get_program()
    return run_bass_kernel_spmd(nc, in_maps, core_ids=list(range(NCORES)),
                                trace=trace, **kw)


def kernel(r, W1, b1, W2, b2, cg, ylm_mix, rf_mix, norm_coef):
    r = np.asarray(r, dtype=np.float32)
    norm_coef_f = np.asarray(norm_coef, dtype=np.float32)
    in_maps = _host_prep(r, W1, b1, W2, b2, cg, ylm_mix, rf_mix, norm_coef_f)
    res = _run_device(in_maps)
    out = np.concatenate(
        [np.asarray(res.results[c]["out"]).astype(np.float32)
         for c in range(NCORES)], axis=0)

    # points with exactly zero radius use norm_coef[..., 1] instead of [..., 0]
    x, y, z = r[:, 0], r[:, 1], r[:, 2]
    r2 = (x * x + y * y) + z * z
    zero = r2 == np.float32(0.0)
    if np.any(zero):
        scale = (norm_coef_f[:, :, 1].astype(np.float64)
                 / norm_coef_f[:, :, 0].astype(np.float64)).reshape(1, IJ)
        out[zero] = (out[zero].astype(np.float64) * scale).astype(np.float32)

    return out.reshape(Z, DO, DI)


# revision 26
# speedup vs baseline: 1.0030x; 1.0001x over previous
"""Trainium2 Bass kernel for the gnn_message_passing problem.

Math refactor: the reference computes
    kernel[z,i,j] = einsum('zk,kij->zij', Rk*Yk, cg) * nc0[i,j]
with Rk = R @ rf_mix.T (rank 6 over paths) and Yk = Y.T @ ylm_mix.T
(rank 9 over l,m).  Rk*Yk has rank <= 54 over k, so the K=1024
contraction folds into one constant matrix
    M[p*9+l, ij] = sum_k rf[k,p] * ylm_s[k,l] * cg[k,ij] * nc0[ij]
and the per-point factor is B[p*9+l, z] = (R+b2)[z,p] * Y'[z,l] -- a
rank-54 stack built from 15 cheap per-point values (6 radial-MLP paths,
9 scaled sh monomials).  B is a pure per-point prefactor (~2% of the
problem's FLOPs) and is prepared host-side in float64 alongside the
constant folds; the device runs the Clebsch-Gordan contraction itself
(98% of FLOPs): per 128-point tile, out = B_tile^T @ M as a k=54 bf16
matmul into f32 PSUM.

The kernel is memory-bound on the output store.  The DMA fabric is a
single ~360 GB/s resource, so bytes are the floor: storing f32 costs
142 us/core.  The output therefore ships as bf16 (25.6 MB/core,
~71 us) and the host widens bf16->f32 during unshard; bf16 rounding
adds <0.5% relative error against the 2e-2 gate.

Per-core pipeline (12500 points = 98 tiles of 128):
  - two merged head loads ([M_lo | B tile 0] and [M_hi | B tiles 1-5])
    on the two lowest-latency trigger paths gate the pipeline ~3.1 us
    in; the remaining B arrives in 4 bulk chunks that all fit inside
    the fill window, so loads cost no store-stream time.
  - per tile: 2 matmuls (B_tile stationary, M halves moving) into two
    INDEPENDENT one-bank PSUM rotations (4 buffers each), so a matmul
    waits only on its own half's drain; the PSUM->SBUF bf16 convert
    splits ACT (first half) + DVE (second half); one 256 KB store,
    alternating SP-HWDGE / Pool-SWDGE triggers so no sequencer or DGE
    saturates.  The first two tiles use quarter-width matmuls and the
    first five use half-column stores (first half Pool-SWDGE, second
    half SP-HWDGE) so the fill phase, which is dispatch-limited, runs
    both store-trigger paths in parallel.

Distribution: data-parallel over z across 8 NeuronCores; constants
replicated.  Full inputs in, full output out.
"""

import numpy as np
import ml_dtypes

import concourse.bass as bass
import concourse.tile as tile
from concourse import bacc, mybir
from concourse.bass_utils import run_bass_kernel_spmd

F32 = mybir.dt.float32
BF16 = mybir.dt.bfloat16

# Problem shape (hardcoded per contract)
Z, KDIM, DO, DI, NPATH, H = 100000, 1024, 32, 32, 6, 128
IJ = DO * DI                      # 1024
PL = NPATH * 9                    # 54 (path x lm)
NCORES = 8
ZC = Z // NCORES                  # 12500 points per core
NT = 98                           # tiles of 128 (12544 >= 12500)
ZPAD = NT * 128

HEAD = 5                          # B tiles packed into the two head loads
CHUNKS = ((5, 24), (29, 23), (52, 23), (75, 23))
SPLIT_MM = 2                      # leading tiles with quarter-width matmuls
HALFCOL = 5                       # leading tiles with half-column stores

# Real spherical harmonic constants (l=0,1,2), folded into M host-side
C0 = 0.28209479177387814
C1 = 0.4886025119029199
C2A = 1.0925484305920792
C2B = 0.31539156525252005
C2C = 0.5462742152960396
YLM_SCALE = np.array([C0, C1, C1, C1, C2A, C2A, C2B, C2A, C2C], dtype=np.float64)

_CACHE = {}


def _build_program():
    nc = bacc.Bacc("TRN2", target_bir_lowering=False, debug=False,
                   num_devices=NCORES)

    hd0 = nc.dram_tensor("hd0", [PL, 512 + 128], BF16,
                         kind="ExternalInput").ap()
    hd1 = nc.dram_tensor("hd1", [PL, 512 + (HEAD - 1) * 128], BF16,
                         kind="ExternalInput").ap()
    bds = [nc.dram_tensor(f"b{i}d", [PL, n * 128], BF16,
                          kind="ExternalInput").ap()
           for i, (_, n) in enumerate(CHUNKS)]
    out = nc.dram_tensor("out", [ZC, IJ], BF16, kind="ExternalOutput").ap()

    with tile.TileContext(nc) as tc:
        with tc.tile_pool(name="const", bufs=1) as cpool, \
             tc.tile_pool(name="kpsA", bufs=4, space="PSUM") as kpoolA, \
             tc.tile_pool(name="kpsB", bufs=4, space="PSUM") as kpoolB, \
             tc.tile_pool(name="kout", bufs=12) as spool:
            h0_sb = cpool.tile([PL, 512 + 128], BF16)
            h1_sb = cpool.tile([PL, 512 + (HEAD - 1) * 128], BF16)
            b_sb = cpool.tile([PL, (NT - HEAD) * 128], BF16)
            mhalves = (h0_sb[:, 0:512], h1_sb[:, 0:512])

            def bt(t):
                if t == 0:
                    return h0_sb[:, 512:640]
                if t < HEAD:
                    return h1_sb[:, 512 + (t - 1) * 128:512 + t * 128]
                return b_sb[:, (t - HEAD) * 128:(t - HEAD + 1) * 128]

            def chunk_load(eng, ci):
                t0, n = CHUNKS[ci]
                c0 = (t0 - HEAD) * 128
                eng.dma_start(b_sb[:, c0:c0 + n * 128], bds[ci][:])

            nc.sync.dma_start(h0_sb[:], hd0[:])
            nc.gpsimd.dma_start(h1_sb[:], hd1[:])
            chunk_load(nc.scalar, 0)
            chunk_load(nc.gpsimd, 1)
            chunk_load(nc.sync, 2)
            chunk_load(nc.scalar, 3)

            for t in range(NT):
                bT = bt(t)
                # independent one-bank PSUM rotations per output half: tile
                # t+4's first matmul can start as soon as tile t's first
                # half-copy (not both) has drained its bank
                kpsA = kpoolA.tile([128, 512], F32, tag="kA")
                kpsB = kpoolB.tile([128, 512], F32, tag="kB")
                halves_ps = (kpsA, kpsB)
                if t < SPLIT_MM:
                    for q in range(4):
                        mq = mhalves[q // 2][:, (q % 2) * 256:
                                             (q % 2 + 1) * 256]
                        nc.tensor.matmul(
                            halves_ps[q // 2][:, (q % 2) * 256:
                                              (q % 2 + 1) * 256],
                            bT, mq, start=True, stop=True)
                else:
                    nc.tensor.matmul(kpsA[:], bT, mhalves[0],
                                     start=True, stop=True)
                    nc.tensor.matmul(kpsB[:], bT, mhalves[1],
                                     start=True, stop=True)
                k_sb = spool.tile([128, IJ], BF16, tag="k_sb")
                nc.scalar.copy(k_sb[:, 0:512], kpsA[:])
                nc.vector.tensor_copy(k_sb[:, 512:1024], kpsB[:])
                zt = t * 128
                rows = min(128, ZC - zt)
                eng = nc.sync if t % 2 == 1 else nc.gpsimd
                if t < HALFCOL:
                    # leading tiles store as halves on split triggers:
                    # first half Pool-SWDGE, second half SP-HWDGE (best
                    # measured fill - the two dispatch paths pipeline)
                    nc.gpsimd.dma_start(out[zt:zt + rows, 0:512],
                                        k_sb[0:rows, 0:512])
                    nc.sync.dma_start(out[zt:zt + rows, 512:1024],
                                      k_sb[0:rows, 512:1024])
                else:
                    eng.dma_start(out[zt:zt + rows, :], k_sb[0:rows, :])
    nc.compile()
    return nc


def _get_program():
    if "nc" not in _CACHE:
        _CACHE["nc"] = _build_program()
    return _CACHE["nc"]


def _host_b(rp):
    """B stack for one core's padded points: B[p*9+l, z] =
    (R[z] + b2)[p] * Y'[z, l], computed in float64, cast to bf16.
    Y' carries the raw monomials; the C-coefficients are folded into M."""
    pts = rp.astype(np.float64)
    x, y, z = pts[:, 0], pts[:, 1], pts[:, 2]
    r2 = x * x + y * y + z * z
    saf = np.where(r2 > 0, r2, 1.0)
    inv_r = 1.0 / np.sqrt(saf)
    inv2 = 1.0 / saf
    radii = r2 * inv_r
    h = np.maximum(radii[:, None] * _CACHE["W1"][0][None, :]
                   + _CACHE["b1"][None, :], 0.0)
    R = h @ _CACHE["W2"] + _CACHE["b2"][None, :]
    yp = np.stack([
        np.ones_like(x), y * inv_r, z * inv_r, x * inv_r,
        x * y * inv2, y * z * inv2, (3.0 * z * z - r2) * inv2,
        x * z * inv2, (x * x - y * y) * inv2,
    ], axis=1)                                            # [z, 9]
    b = (R[:, :, None] * yp[:, None, :]).reshape(-1, PL)  # [z, 54]
    return np.ascontiguousarray(b.T).astype(ml_dtypes.bfloat16)


def _host_prep(r, W1, b1, W2, b2, cg, ylm_mix, rf_mix, norm_coef):
    r = np.asarray(r, dtype=np.float32)
    W1 = np.asarray(W1, dtype=np.float32)
    b1 = np.asarray(b1, dtype=np.float32)
    W2 = np.asarray(W2, dtype=np.float32)
    b2 = np.asarray(b2, dtype=np.float32)
    cg = np.asarray(cg, dtype=np.float32)
    ylm_mix = np.asarray(ylm_mix, dtype=np.float32)
    rf_mix = np.asarray(rf_mix, dtype=np.float32)
    norm_coef = np.asarray(norm_coef, dtype=np.float32)
    _CACHE["W1"] = W1.astype(np.float64)
    _CACHE["b1"] = b1.astype(np.float64)
    _CACHE["W2"] = W2.astype(np.float64)
    _CACHE["b2"] = b2.astype(np.float64)

    # Fold the constant k-contraction: M[p*9+l, ij] =
    #   sum_k rf[k,p] * (ylm[k,l]*scale_l) * cg[k,ij], times nc0[ij]
    ylm_s = ylm_mix.astype(np.float64) * YLM_SCALE[None, :]
    w54 = (rf_mix.astype(np.float64)[:, :, None]
           * ylm_s[:, None, :]).reshape(KDIM, PL)
    mfold = w54.T @ cg.astype(np.float64).reshape(KDIM, IJ)
    mfold *= norm_coef[:, :, 0].astype(np.float64).reshape(1, IJ)
    mn = mfold.astype(ml_dtypes.bfloat16)

    in_maps = []
    for c in range(NCORES):
        rs = r[c * ZC:(c + 1) * ZC]
        rp = np.empty((ZPAD, 3), dtype=np.float32)
        rp[:ZC] = rs
        rp[ZC:] = np.array([1.0, 0.0, 0.0], dtype=np.float32)
        bfull = _host_b(rp)                               # [54, ZPAD] bf16
        h0 = np.concatenate([mn[:, 0:512], bfull[:, 0:128]], axis=1)
        h1 = np.concatenate([mn[:, 512:1024],
                             bfull[:, 128:HEAD * 128]], axis=1)
        m = {"hd0": np.ascontiguousarray(h0), "hd1": np.ascontiguousarray(h1)}
        for i, (t0, n) in enumerate(CHUNKS):
            m[f"b{i}d"] = np.ascontiguousarray(
                bfull[:, t0 * 128:(t0 + n) * 128])
        in_maps.append(m)
    return in_maps


def _run_device(in_maps, trace=False, **kw):
    nc = _get_program()
    return run_bass_kernel_spmd(nc, in_maps, core_ids=list(range(NCORES)),
                                trace=trace, **kw)


def kernel(r, W1, b1, W2, b2, cg, ylm_mix, rf_mix, norm_coef):
    r = np.asarray(r, dtype=np.float32)
    norm_coef_f = np.asarray(norm_coef, dtype=np.float32)
    in_maps = _host_prep(r, W1, b1, W2, b2, cg, ylm_mix, rf_mix, norm_coef_f)
    res = _run_device(in_maps)
    out = np.concatenate(
        [np.asarray(res.results[c]["out"]).astype(np.float32)
         for c in range(NCORES)], axis=0)

    # points with exactly zero radius use norm_coef[..., 1] instead of [..., 0]
    x, y, z = r[:, 0], r[:, 1], r[:, 2]
    r2 = (x * x + y * y) + z * z
    zero = r2 == np.float32(0.0)
    if np.any(zero):
        scale = (norm_coef_f[:, :, 1].astype(np.float64)
                 / norm_coef_f[:, :, 0].astype(np.float64)).reshape(1, IJ)
        out[zero] = (out[zero].astype(np.float64) * scale).astype(np.float32)

    return out.reshape(Z, DO, DI)


# revision 28
# speedup vs baseline: 1.0030x; 1.0001x over previous
"""Trainium2 Bass kernel for the gnn_message_passing problem.

Math refactor: the reference computes
    kernel[z,i,j] = einsum('zk,kij->zij', Rk*Yk, cg) * nc0[i,j]
with Rk = R @ rf_mix.T (rank 6 over paths) and Yk = Y.T @ ylm_mix.T
(rank 9 over l,m).  Rk*Yk has rank <= 54 over k, so the K=1024
contraction folds into one constant matrix
    M[p*9+l, ij] = sum_k rf[k,p] * ylm_s[k,l] * cg[k,ij] * nc0[ij]
and the per-point factor is B[p*9+l, z] = (R+b2)[z,p] * Y'[z,l] -- a
rank-54 stack built from 15 cheap per-point values (6 radial-MLP paths,
9 scaled sh monomials).  B is a pure per-point prefactor (~2% of the
problem's FLOPs) and is prepared host-side in float64 alongside the
constant folds; the device runs the Clebsch-Gordan contraction itself
(98% of FLOPs): per 128-point tile, out = B_tile^T @ M as a k=54 bf16
matmul into f32 PSUM.

The kernel is memory-bound on the output store.  The DMA fabric is a
single ~360 GB/s resource, so bytes are the floor: storing f32 costs
142 us/core.  The output therefore ships as bf16 (25.6 MB/core,
~71 us) and the host widens bf16->f32 during unshard; bf16 rounding
adds <0.5% relative error against the 2e-2 gate.

Per-core pipeline (12500 points = 98 tiles of 128):
  - two merged head loads ([M_lo | B tile 0] and [M_hi | B tiles 1-5])
    on the two lowest-latency trigger paths gate the pipeline ~3.1 us
    in; the remaining B arrives in 4 bulk chunks that all fit inside
    the fill window, so loads cost no store-stream time.
  - per tile: 2 matmuls (B_tile stationary, M halves moving) into two
    INDEPENDENT one-bank PSUM rotations (4 buffers each), so a matmul
    waits only on its own half's drain; the PSUM->SBUF bf16 convert
    splits ACT (first half) + DVE (second half); one 256 KB store,
    alternating SP-HWDGE / Pool-SWDGE triggers so no sequencer or DGE
    saturates.  The first two tiles use quarter-width matmuls and the
    first five use half-column stores (first half Pool-SWDGE, second
    half SP-HWDGE) so the fill phase, which is dispatch-limited, runs
    both store-trigger paths in parallel.

Distribution: data-parallel over z across 8 NeuronCores; constants
replicated.  Full inputs in, full output out.
"""

import numpy as np
import ml_dtypes

import concourse.bass as bass
import concourse.tile as tile
from concourse import bacc, mybir
from concourse.bass_utils import run_bass_kernel_spmd

F32 = mybir.dt.float32
BF16 = mybir.dt.bfloat16

# Problem shape (hardcoded per contract)
Z, KDIM, DO, DI, NPATH, H = 100000, 1024, 32, 32, 6, 128
IJ = DO * DI                      # 1024
PL = NPATH * 9                    # 54 (path x lm)
NCORES = 8
ZC = Z // NCORES                  # 12500 points per core
NT = 98                           # tiles of 128 (12544 >= 12500)
ZPAD = NT * 128

HEAD = 5                          # B tiles packed into the two head loads
CHUNKS = ((5, 24), (29, 23), (52, 23), (75, 23))
SPLIT_MM = 2                      # leading tiles with quarter-width matmuls
HALFCOL = 5                       # leading tiles with half-column stores

# Real spherical harmonic constants (l=0,1,2), folded into M host-side
C0 = 0.28209479177387814
C1 = 0.4886025119029199
C2A = 1.0925484305920792
C2B = 0.31539156525252005
C2C = 0.5462742152960396
YLM_SCALE = np.array([C0, C1, C1, C1, C2A, C2A, C2B, C2A, C2C], dtype=np.float64)

_CACHE = {}


def _build_program():
    nc = bacc.Bacc("TRN2", target_bir_lowering=False, debug=False,
                   num_devices=NCORES)

    hd0 = nc.dram_tensor("hd0", [PL, 512 + 128], BF16,
                         kind="ExternalInput").ap()
    hd1 = nc.dram_tensor("hd1", [PL, 512 + (HEAD - 1) * 128], BF16,
                         kind="ExternalInput").ap()
    bds = [nc.dram_tensor(f"b{i}d", [PL, n * 128], BF16,
                          kind="ExternalInput").ap()
           for i, (_, n) in enumerate(CHUNKS)]
    out = nc.dram_tensor("out", [ZC, IJ], BF16, kind="ExternalOutput").ap()

    with tile.TileContext(nc) as tc:
        with tc.tile_pool(name="const", bufs=1) as cpool, \
             tc.tile_pool(name="kpsA", bufs=4, space="PSUM") as kpoolA, \
             tc.tile_pool(name="kpsB", bufs=4, space="PSUM") as kpoolB, \
             tc.tile_pool(name="kout", bufs=12) as spool:
            h0_sb = cpool.tile([PL, 512 + 128], BF16)
            h1_sb = cpool.tile([PL, 512 + (HEAD - 1) * 128], BF16)
            b_sb = cpool.tile([PL, (NT - HEAD) * 128], BF16)
            mhalves = (h0_sb[:, 0:512], h1_sb[:, 0:512])

            def bt(t):
                if t == 0:
                    return h0_sb[:, 512:640]
                if t < HEAD:
                    return h1_sb[:, 512 + (t - 1) * 128:512 + t * 128]
                return b_sb[:, (t - HEAD) * 128:(t - HEAD + 1) * 128]

            # the last 44 B columns are padding past the 12500 real points:
            # skip loading them (saves DMA stream time) and zero-fill the
            # SBUF tail instead so tile 97's matmul reads finite values
            TRIM = NT * 128 - ZC

            def chunk_load(eng, ci):
                t0, n = CHUNKS[ci]
                c0 = (t0 - HEAD) * 128
                w = n * 128 - (TRIM if ci == len(CHUNKS) - 1 else 0)
                eng.dma_start(b_sb[:, c0:c0 + w], bds[ci][:, 0:w])

            nc.sync.dma_start(h0_sb[:], hd0[:])
            nc.gpsimd.dma_start(h1_sb[:], hd1[:])
            chunk_load(nc.scalar, 0)
            chunk_load(nc.gpsimd, 1)
            chunk_load(nc.sync, 2)
            chunk_load(nc.scalar, 3)

            for t in range(NT):
                bT = bt(t)
                # independent one-bank PSUM rotations per output half: tile
                # t+4's first matmul can start as soon as tile t's first
                # half-copy (not both) has drained its bank
                kpsA = kpoolA.tile([128, 512], F32, tag="kA")
                kpsB = kpoolB.tile([128, 512], F32, tag="kB")
                halves_ps = (kpsA, kpsB)
                if t < SPLIT_MM:
                    for q in range(4):
                        mq = mhalves[q // 2][:, (q % 2) * 256:
                                             (q % 2 + 1) * 256]
                        nc.tensor.matmul(
                            halves_ps[q // 2][:, (q % 2) * 256:
                                              (q % 2 + 1) * 256],
                            bT, mq, start=True, stop=True)
                else:
                    nc.tensor.matmul(kpsA[:], bT, mhalves[0],
                                     start=True, stop=True)
                    nc.tensor.matmul(kpsB[:], bT, mhalves[1],
                                     start=True, stop=True)
                k_sb = spool.tile([128, IJ], BF16, tag="k_sb")
                nc.scalar.copy(k_sb[:, 0:512], kpsA[:])
                nc.vector.tensor_copy(k_sb[:, 512:1024], kpsB[:])
                if t == 50:
                    # zero the unloaded B tail mid-stream: DVE has queue
                    # slack here, and it completes long before tile 97
                    # reads it
                    nc.vector.memset(
                        b_sb[:, (NT - HEAD) * 128 - TRIM:].bitcast(
                            mybir.dt.uint16), 0)
                zt = t * 128
                rows = min(128, ZC - zt)
                eng = nc.sync if t % 2 == 1 else nc.gpsimd
                if t < HALFCOL:
                    # leading tiles store as halves on split triggers:
                    # first half Pool-SWDGE, second half SP-HWDGE (best
                    # measured fill - the two dispatch paths pipeline)
                    nc.gpsimd.dma_start(out[zt:zt + rows, 0:512],
                                        k_sb[0:rows, 0:512])
                    nc.sync.dma_start(out[zt:zt + rows, 512:1024],
                                      k_sb[0:rows, 512:1024])
                else:
                    eng.dma_start(out[zt:zt + rows, :], k_sb[0:rows, :])
    nc.compile()
    return nc


def _get_program():
    if "nc" not in _CACHE:
        _CACHE["nc"] = _build_program()
    return _CACHE["nc"]


def _host_b(rp):
    """B stack for one core's padded points: B[p*9+l, z] =
    (R[z] + b2)[p] * Y'[z, l], computed in float64, cast to bf16.
    Y' carries the raw monomials; the C-coefficients are folded into M."""
    pts = rp.astype(np.float64)
    x, y, z = pts[:, 0], pts[:, 1], pts[:, 2]
    r2 = x * x + y * y + z * z
    saf = np.where(r2 > 0, r2, 1.0)
    inv_r = 1.0 / np.sqrt(saf)
    inv2 = 1.0 / saf
    radii = r2 * inv_r
    h = np.maximum(radii[:, None] * _CACHE["W1"][0][None, :]
                   + _CACHE["b1"][None, :], 0.0)
    R = h @ _CACHE["W2"] + _CACHE["b2"][None, :]
    yp = np.stack([
        np.ones_like(x), y * inv_r, z * inv_r, x * inv_r,
        x * y * inv2, y * z * inv2, (3.0 * z * z - r2) * inv2,
        x * z * inv2, (x * x - y * y) * inv2,
    ], axis=1)                                            # [z, 9]
    b = (R[:, :, None] * yp[:, None, :]).reshape(-1, PL)  # [z, 54]
    return np.ascontiguousarray(b.T).astype(ml_dtypes.bfloat16)


def _host_prep(r, W1, b1, W2, b2, cg, ylm_mix, rf_mix, norm_coef):
    r = np.asarray(r, dtype=np.float32)
    W1 = np.asarray(W1, dtype=np.float32)
    b1 = np.asarray(b1, dtype=np.float32)
    W2 = np.asarray(W2, dtype=np.float32)
    b2 = np.asarray(b2, dtype=np.float32)
    cg = np.asarray(cg, dtype=np.float32)
    ylm_mix = np.asarray(ylm_mix, dtype=np.float32)
    rf_mix = np.asarray(rf_mix, dtype=np.float32)
    norm_coef = np.asarray(norm_coef, dtype=np.float32)
    _CACHE["W1"] = W1.astype(np.float64)
    _CACHE["b1"] = b1.astype(np.float64)
    _CACHE["W2"] = W2.astype(np.float64)
    _CACHE["b2"] = b2.astype(np.float64)

    # Fold the constant k-contraction: M[p*9+l, ij] =
    #   sum_k rf[k,p] * (ylm[k,l]*scale_l) * cg[k,ij], times nc0[ij]
    ylm_s = ylm_mix.astype(np.float64) * YLM_SCALE[None, :]
    w54 = (rf_mix.astype(np.float64)[:, :, None]
           * ylm_s[:, None, :]).reshape(KDIM, PL)
    mfold = w54.T @ cg.astype(np.float64).reshape(KDIM, IJ)
    mfold *= norm_coef[:, :, 0].astype(np.float64).reshape(1, IJ)
    mn = mfold.astype(ml_dtypes.bfloat16)

    in_maps = []
    for c in range(NCORES):
        rs = r[c * ZC:(c + 1) * ZC]
        rp = np.empty((ZPAD, 3), dtype=np.float32)
        rp[:ZC] = rs
        rp[ZC:] = np.array([1.0, 0.0, 0.0], dtype=np.float32)
        bfull = _host_b(rp)                               # [54, ZPAD] bf16
        h0 = np.concatenate([mn[:, 0:512], bfull[:, 0:128]], axis=1)
        h1 = np.concatenate([mn[:, 512:1024],
                             bfull[:, 128:HEAD * 128]], axis=1)
        m = {"hd0": np.ascontiguousarray(h0), "hd1": np.ascontiguousarray(h1)}
        for i, (t0, n) in enumerate(CHUNKS):
            m[f"b{i}d"] = np.ascontiguousarray(
                bfull[:, t0 * 128:(t0 + n) * 128])
        in_maps.append(m)
    return in_maps


def _run_device(in_maps, trace=False, **kw):
    nc = _get_program()
    return run_bass_kernel_spmd(nc, in_maps, core_ids=list(range(NCORES)),
                                trace=trace, **kw)


def kernel(r, W1, b1, W2, b2, cg, ylm_mix, rf_mix, norm_coef):
    r = np.asarray(r, dtype=np.float32)
    norm_coef_f = np.asarray(norm_coef, dtype=np.float32)
    in_maps = _host_prep(r, W1, b1, W2, b2, cg, ylm_mix, rf_mix, norm_coef_f)
    res = _run_device(in_maps)
    out = np.concatenate(
        [np.asarray(res.results[c]["out"]).astype(np.float32)
         for c in range(NCORES)], axis=0)

    # points with exactly zero radius use norm_coef[..., 1] instead of [..., 0]
    x, y, z = r[:, 0], r[:, 1], r[:, 2]
    r2 = (x * x + y * y) + z * z
    zero = r2 == np.float32(0.0)
    if np.any(zero):
        scale = (norm_coef_f[:, :, 1].astype(np.float64)
                 / norm_coef_f[:, :, 0].astype(np.float64)).reshape(1, IJ)
        out[zero] = (out[zero].astype(np.float64) * scale).astype(np.float32)

    return out.reshape(Z, DO, DI)
